# revision 16
# baseline (speedup 1.0000x reference)
"""CapsuleLayer dynamic-routing kernel for 8 Trainium2 NeuronCores.

Math (reference):
    u_hat[b,n,j,d] = sum_i W[n,j,d,i] * x[b,j,i]
    b = 0; for r in 0..2:
        c = softmax_n(b); s[b,n,d] = sum_j c*u_hat; v = squash_d(s)
        if r < 2: b += sum_d v*u_hat
    return v  [B, N, D]

Sharding: J (input capsules, 2048) split 8 ways -> Jc=256 per core.
Softmax over n is local; only s needs a 256 KiB AllReduce per iteration.

Per-core design (v2):
  r0: c uniform -> s0 = (1/N) sum_{j,i} x*W, computed as K=128 matmuls
      contracting (j8,i) chunks; W streamed in layout-A [ (j8,i), (n,d) ]
      across all 128 partitions (fast DMA). No vector work.
  r1/r2: j in groups of 4, group j's placed on DIAGONAL PE tiles
      (row band 32a holds j=4g+a at partitions 32a..32a+16) so W tiles
      span 128 partitions (fast DMA, 2 MiB per 4-group supergroup).
      Per group: u in PSUM -> scalar evac to bf16 -> DVE tl = u*v_rep
      -> bf16 pair-add over d (L1) -> reduce over 8? (16) -> logits.
      Per supergroup: batched exp with PAIR-DUPLICATED output e2,
      zsum on gpsimd, zinv folded into the s-matmul stationary
      (w4 = 2*I * zinv, the x2 pre-folded into host ones4), so
      t2 = e2 * u directly (no c normalization pass).
      s accumulated over all groups in one PSUM bank via w4-matmuls.
  AllReduce s in fp32 via DRAM bounce, squash redundantly per core.
"""

import functools
import numpy as np

B, J, I = 32, 2048, 16
N, D = 64, 32
NCORES = 8
JC = J // NCORES          # 256 j per core
GRP = 4                   # j's per group (PE diagonal bands)
NG = JC // GRP            # 64 groups
SG = 4                    # groups per supergroup
NSG = NG // SG            # 16 supergroups
NCH = JC // 8             # 32 K=128 chunks for r0
ND = N * D                # 2048
HALF = ND // 2            # 1024
ROUTINGS = 3
EPS = 1e-7

# engine assignment knobs
GS_REDUCE = False         # gpsimd tensor_reduce can't do free-dim reductions
GS_T2 = 3                 # how many of each supergroup's 4 t2 ops go to gpsimd


@functools.lru_cache(maxsize=1)
def _build():
    import concourse.bass as bass
    import concourse.mybir as mybir
    import concourse.bacc as bacc
    import concourse.tile as tile

    f32 = mybir.dt.float32
    bf16 = mybir.dt.bfloat16
    MUL = mybir.AluOpType.mult
    ADD = mybir.AluOpType.add
    AX = mybir.AxisListType.X
    AF = mybir.ActivationFunctionType

    nc = bacc.Bacc("TRN2", target_bir_lowering=False, debug=False,
                   num_devices=NCORES)

    xa_d = nc.dram_tensor("xa", [128, NCH, B], bf16, kind="ExternalInput")
    wa_d = nc.dram_tensor("wa", [128, NCH, ND], bf16, kind="ExternalInput")
    xb_d = nc.dram_tensor("xb", [128, NG, B], bf16, kind="ExternalInput")
    wb_d = nc.dram_tensor("wb", [NSG, 128, SG * ND], bf16, kind="ExternalInput")
    ones_d = nc.dram_tensor("ones4", [128, B], bf16, kind="ExternalInput")
    v_d = nc.dram_tensor("v", [B, ND], f32, kind="ExternalOutput")

    with tile.TileContext(nc) as tc:
        with (
            tc.tile_pool(name="persist", bufs=1) as pp,
            tc.tile_pool(name="wstream", bufs=2) as wp,
            tc.tile_pool(name="usb", bufs=10) as up,
            tc.tile_pool(name="tl", bufs=3) as tp,
            tc.tile_pool(name="tl1", bufs=3) as t1p,
            tc.tile_pool(name="t2", bufs=3) as t2p,
            tc.tile_pool(name="small", bufs=2) as sm,
            tc.tile_pool(name="w4", bufs=6) as w4p,
            tc.tile_pool(name="ups", bufs=3, space="PSUM") as ups_pool,
            tc.tile_pool(name="sps", bufs=1, space="PSUM") as sps_pool,
            tc.tile_pool(name="dram", bufs=1, space="DRAM") as dr,
        ):
            xa = pp.tile([128, NCH, B], bf16)
            nc.sync.dma_start(xa[:], xa_d[:])
            xb = pp.tile([128, NG, B], bf16)
            nc.sync.dma_start(xb[:], xb_d[:])
            ones4 = pp.tile([128, B], bf16)
            nc.sync.dma_start(ones4[:], ones_d[:])

            logits = pp.tile([128, NG, N], bf16)
            v_rep = pp.tile([128, ND], bf16)
            v_small = pp.tile([B, ND], bf16)
            s_sb = pp.tile([128, 512], f32)
            v_sb = pp.tile([B, ND], f32)
            s0 = pp.tile([B, ND], f32)
            s_evac = pp.tile([128, 512], f32)

            cc_in = dr.tile([128, 512], f32)
            cc_out = dr.tile([128, 512], f32)

            def cc_and_squash(r):
                """AllReduce cc_in -> cc_out, squash in quarter-strip
                layout (partition 32q+b holds n in [16q,16q+16), all d),
                write v_rep (r<2) or v_sb (r=2)."""
                nc.gpsimd.collective_compute(
                    "AllReduce", ADD,
                    replica_groups=[list(range(NCORES))],
                    ins=[cc_in[:].opt()], outs=[cc_out[:].opt()],
                )
                nc.sync.dma_start(s_sb[:], cc_out[:])

                sq = sm.tile([128, 16, D], f32)
                s3 = s_sb[:].rearrange("p (n d) -> p n d", d=D)
                nc.vector.tensor_tensor(sq[:], s3, s3, op=MUL)
                ns2 = sm.tile([128, 16], f32)
                nc.vector.tensor_reduce(ns2[:], sq[:], axis=AX, op=ADD)
                onep = sm.tile([128, 16], f32)
                nc.vector.tensor_scalar_add(onep[:], ns2[:], 1.0)
                rt = sm.tile([128, 16], f32)
                eps_t = sm.tile([128, 1], f32)
                nc.vector.memset(eps_t[:], EPS)
                nc.scalar.activation(rt[:], ns2[:], AF.Sqrt, bias=eps_t[:])
                den = sm.tile([128, 16], f32)
                nc.vector.tensor_tensor(den[:], onep[:], rt[:], op=MUL)
                dinv = sm.tile([128, 16], f32)
                nc.vector.reciprocal(dinv[:], den[:])
                scl = sm.tile([128, 16], f32)
                nc.vector.tensor_tensor(scl[:], ns2[:], dinv[:], op=MUL)
                v4 = sm.tile([128, 16, D], f32)
                nc.vector.tensor_tensor(
                    v4[:], s3,
                    scl[:, :, None].broadcast_to([128, 16, D]),
                    op=MUL)

                if r < ROUTINGS - 1:
                    v4b = sm.tile([128, 512], bf16)
                    nc.vector.tensor_copy(
                        v4b[:], v4[:].rearrange("p a b -> p (a b)"))
                    for q in range(4):
                        nc.sync.dma_start(
                            v_small[:, q * 512:(q + 1) * 512],
                            v4b[32 * q:32 * q + 32, :])
                    for rr in range(GRP):
                        nc.sync.dma_start(
                            v_rep[32 * rr:32 * rr + 32, :],
                            v_small[:])
                else:
                    for q in range(4):
                        nc.sync.dma_start(
                            v_sb[:, q * 512:(q + 1) * 512],
                            v4[32 * q:32 * q + 32, :])

            # ---------------- r0: uniform c ----------------
            # s0*N = sum_{(j,i)} xa[(j8,i),ch,b] * wa[ch,(j8,i),(n,d)]
            # (1/N folded into host xa values)
            acc = [ups_pool.tile([128, HALF], f32, name="u_ps", tag="ups")
                   for _h in range(2)]
            CHB = 4  # chunks per DMA (1 MiB used of 2 MiB tile)
            for cb in range(NCH // CHB):
                wt = wp.tile([128, CHB * ND], bf16, name="w_t", tag="wst")
                nc.sync.dma_start(
                    wt[:], wa_d[:, cb * CHB:(cb + 1) * CHB, :])
                wt = wt[:].rearrange("p (c f) -> p c f", c=CHB)
                for cc_ in range(CHB):
                    ch = cb * CHB + cc_
                    for h in range(2):
                        for q in range(2):
                            nc.tensor.matmul(
                                acc[h][0:B, q * 512:(q + 1) * 512],
                                xa[:, ch, :],
                                wt[:, cc_, h * HALF + q * 512:
                                    h * HALF + (q + 1) * 512],
                                start=(ch == 0), stop=(ch == NCH - 1),
                                skip_group_check=True,
                            )
            for h in range(2):
                nc.scalar.activation(
                    s0[:, h * HALF:(h + 1) * HALF], acc[h][0:B, :], AF.Copy)
            for q in range(4):
                nc.sync.dma_start(
                    cc_in[32 * q:32 * q + 32, :],
                    s0[:, q * 512:(q + 1) * 512])
            cc_and_squash(0)

            # ---------------- r1, r2 ----------------
            for r in range(1, ROUTINGS):
                s_ps = sps_pool.tile([128, 512], f32)
                wb_r = wb_d  # alias

                state = []  # pending supergroup: (sg, u_sbs)

                def u_phase(sg):
                    wt = wp.tile([128, SG * ND], bf16, name="w_t", tag="wst")
                    nc.sync.dma_start(wt[:], wb_r[sg, :, :])
                    wtv = wt[:].rearrange("p (g f) -> p g f", g=SG)
                    u_sbs = []
                    for gi in range(SG):
                        g = sg * SG + gi
                        u_sb = up.tile([128, ND], bf16, name="u_sb", tag="usb")
                        for h in range(2):
                            u_ps = ups_pool.tile([128, HALF], f32,
                                                 name="u_ps", tag="ups")
                            for a in range(GRP):
                                for q in range(2):
                                    nc.tensor.matmul(
                                        u_ps[32 * a:32 * a + 32,
                                             q * 512:(q + 1) * 512],
                                        xb[32 * a:32 * a + 16, g, :],
                                        wtv[32 * a:32 * a + 16, gi,
                                            h * HALF + q * 512:
                                            h * HALF + (q + 1) * 512],
                                        start=True, stop=True,
                                        tile_position=(32 * a, 32 * a),
                                        skip_group_check=True,
                                    )
                            nc.scalar.activation(
                                u_sb[:, h * HALF:(h + 1) * HALF],
                                u_ps[:], AF.Copy)
                        # tl = u * v_rep  (flat 2D, bf16, 2x mode)
                        tl = tp.tile([128, ND], bf16, name="tl", tag="tl")
                        nc.vector.tensor_tensor(tl[:], u_sb[:], v_rep[:],
                                                op=MUL)
                        tl3 = tl[:].rearrange("p (n d) -> p n d", d=D)
                        tl1 = t1p.tile([128, N, 16], bf16, name="tl1",
                                       tag="tl1")
                        with nc.allow_low_precision("bf16 logits pyramid"):
                            nc.vector.tensor_tensor(
                                tl1[:], tl3[:, :, 0:16], tl3[:, :, 16:32],
                                op=ADD)
                            red_eng = nc.gpsimd if GS_REDUCE else nc.vector
                            if r == 1:
                                red_eng.tensor_reduce(
                                    logits[:, g, :], tl1[:], axis=AX, op=ADD)
                            else:
                                dtmp = sm.tile([128, N], bf16, name="dtmp",
                                               tag="dtmp")
                                red_eng.tensor_reduce(
                                    dtmp[:], tl1[:], axis=AX, op=ADD)
                                nc.vector.tensor_add(
                                    logits[:, g, :], logits[:, g, :],
                                    dtmp[:])
                        u_sbs.append(u_sb)
                    return u_sbs

                def s_phase(sg, u_sbs):
                    sl = slice(sg * SG, (sg + 1) * SG)
                    # batched softmax pieces: e2 = exp(logits), pair-dup
                    e2 = sm.tile([128, SG, N, 2], bf16, name="e2", tag="e2")
                    nc.scalar.activation(
                        e2[:], logits[:, sl, :, None]
                        .broadcast_to([128, SG, N, 2]), AF.Exp)
                    zsum = sm.tile([128, SG], f32, name="zs", tag="zs")
                    nc.vector.tensor_reduce(
                        zsum[:], e2[:].rearrange("p g n t -> p g (n t)"),
                        axis=AX, op=ADD)
                    zrec = sm.tile([128, SG], f32, name="zr", tag="zr")
                    nc.vector.reciprocal(zrec[:], zsum[:])
                    for gi in range(SG):
                        g = sg * SG + gi
                        w4 = w4p.tile([128, B], bf16, name="w4", tag="w4")
                        with nc.allow_low_precision("bf16 zinv weights"):
                            nc.vector.tensor_scalar_mul(
                                w4[:], ones4[:], zrec[:, gi:gi + 1])
                        t2 = t2p.tile([128, ND], bf16, name="t2", tag="t2")
                        eng = nc.gpsimd if gi < GS_T2 else nc.vector
                        eng.tensor_tensor(
                            t2[:].rearrange("p (n a t) -> p n a t",
                                            n=N, a=16),
                            u_sbs[gi][:].rearrange("p (n a t) -> p n a t",
                                                   n=N, a=16),
                            e2[:, gi, :, None, :]
                            .broadcast_to([128, N, 16, 2]),
                            op=MUL)
                        for q in range(4):
                            nc.tensor.matmul(
                                s_ps[32 * q:32 * q + 32, :],
                                w4[:],
                                t2[:, q * 512:(q + 1) * 512],
                                start=(g == 0), stop=(g == NG - 1),
                                tile_position=(0, 32 * q),
                                skip_group_check=True,
                            )

                for sg in range(NSG):
                    u_sbs = u_phase(sg)
                    if state:
                        s_phase(*state.pop(0))
                    state.append((sg, u_sbs))
                while state:
                    s_phase(*state.pop(0))

                nc.vector.tensor_copy(s_evac[:], s_ps[:])
                nc.sync.dma_start(cc_in[:], s_evac[:])
                cc_and_squash(r)

            nc.sync.dma_start(v_d[:], v_sb[:])

    nc.compile()
    return nc


def prepare_inputs(x: np.ndarray, W: np.ndarray):
    """Full inputs -> per-core input maps (host-side reshuffles)."""
    import ml_dtypes
    bf = ml_dtypes.bfloat16

    ones4 = np.tile(2.0 * np.eye(B, dtype=np.float32), (GRP, 1)).astype(bf)

    in_maps = []
    for k in range(NCORES):
        jlo, jhi = k * JC, (k + 1) * JC
        Wc = W[:, jlo:jhi]                       # [N, JC, D, I]
        xc = x[:, jlo:jhi]                       # [B, JC, I]
        arrw = np.ascontiguousarray(
            Wc.transpose(1, 3, 0, 2)).reshape(JC, I, ND)   # [j, i, (n,d)]
        arrx = np.ascontiguousarray(xc.transpose(1, 2, 0))  # [j, i, b]

        wa = arrw.reshape(NCH, 8, I, ND).transpose(1, 2, 0, 3) \
            .reshape(128, NCH, ND)
        xa = (arrx / N).reshape(NCH, 8, I, B).transpose(1, 2, 0, 3) \
            .reshape(128, NCH, B)

        wb4 = arrw.reshape(NSG, SG, GRP, I, ND)   # [sg, gi, a, i, nd]
        wbp = np.zeros((NSG, GRP, 32, SG, ND), dtype=np.float32)
        wbp[:, :, :I] = wb4.transpose(0, 2, 3, 1, 4)  # [sg, a, i, gi, nd]
        wb = wbp.reshape(NSG, 128, SG * ND)

        xb4 = arrx.reshape(NG, GRP, I, B)         # [g, a, i, b]
        xbp = np.zeros((GRP, 32, NG, B), dtype=np.float32)
        xbp[:, :I] = xb4.transpose(1, 2, 0, 3)    # [a, i, g, b]
        xb = xbp.reshape(128, NG, B)

        in_maps.append({
            "xa": np.ascontiguousarray(xa).astype(bf),
            "wa": np.ascontiguousarray(wa).astype(bf),
            "xb": np.ascontiguousarray(xb).astype(bf),
            "wb": np.ascontiguousarray(wb).astype(bf),
            "ones4": ones4,
        })
    return in_maps


def kernel(x: np.ndarray, W: np.ndarray) -> np.ndarray:
    from concourse.bass_utils import run_bass_kernel_spmd

    nc = _build()
    in_maps = prepare_inputs(x, W)
    res = run_bass_kernel_spmd(nc, in_maps, list(range(NCORES)))
    v = np.asarray(res.results[0]["v"], dtype=np.float32)
    return v.reshape(B, N, D)


if __name__ == "__main__":
    rng = np.random.default_rng(0)
    x = rng.normal(size=(B, J, I)).astype(np.float32)
    W = rng.normal(size=(N, J, D, I)).astype(np.float32) * 0.05
    v = kernel(x, W)
    print(v.shape, v.dtype, np.abs(v).max())
